# revision 28
# baseline (speedup 1.0000x reference)
"""Talking-heads attention kernel for Trainium2, 8 NeuronCores.

Problem: B=4, N=2048, DIM=512, H=8, DH=64 talking-heads attention
(qkv proj -> per-head scores -> th1 head-mix -> softmax -> th2 head-mix
 -> attn @ v -> out proj).

Sharding: data-parallel over (batch, query-half): core c handles batch c//2,
query rows [1024*(c%2), 1024*(c%2)+1024).  Communication-free.  The host
rolls each core's copy of x so its own query rows come first; key order is
a permutation, which attention is invariant to (K and V share it).

Device pipeline per core (all matmuls bf16 with fp32 PSUM accumulation):
  1. x arrives bf16; PE identity-matmul transposes build x^T (the xbar is
     avoided in phase A: its descriptor flood serializes with the HBM
     loads); QKV projection on TensorE producing Q^T/K^T and V (n-major).
  2. Per 16-query group j: th1-fused scores land directly in
     head-interleaved PSUM [(g,n16), m] (qhat columns are th1-scaled).
  3. exp on ScalarE with fused per-row accumulation; the l2 stationary
     (th2[g,h]/Z row scaling) is built immediately after the exps so the
     mix2 matmuls never head-of-line-block the PE queue.
  4. mix2 output is xbar-DMA-transposed to key-major A^T, attn@V runs with
     V as the stationary operand, followed by the output projection.
"""

import sys

sys.path.insert(0, "/opt/trn_rl_repo")

import numpy as np
import ml_dtypes

import concourse.bass as bass
from concourse import bacc
import concourse.mybir as mybir
import concourse.tile as tile
from concourse.bass_utils import run_bass_kernel_spmd

BF16 = mybir.dt.bfloat16
F32 = mybir.dt.float32
AF = mybir.ActivationFunctionType

B, N, DIM = 4, 2048, 512
H, DH = 8, 64
NCORES = 8
NLOC = N // 2          # query rows per core
NT = NLOC // 128       # 8 query tiles per core
MT = N // 128          # 16 key chunks
NG = 16                # n16 group size in interleaved tiles


def build_nc():
    nc = bacc.Bacc()

    x = nc.declare_dram_parameter("x", [N, DIM], BF16, isOutput=False)
    wq = nc.declare_dram_parameter("wq", [DIM, DIM], BF16, isOutput=False)
    wk = nc.declare_dram_parameter("wk", [DIM, DIM], BF16, isOutput=False)
    wv = nc.declare_dram_parameter("wv", [DIM, DIM], BF16, isOutput=False)
    wo = nc.declare_dram_parameter("wo", [DIM, DIM], BF16, isOutput=False)
    th1s = nc.declare_dram_parameter("th1s", [128, 32], F32, isOutput=False)
    t2t = nc.declare_dram_parameter("t2t", [128, 128], BF16, isOutput=False)
    # rows: bq (pre-scaled), bk, bv, bo
    bia = nc.declare_dram_parameter("bia", [1, 4 * DIM], BF16, isOutput=False)
    eye = nc.declare_dram_parameter("eye", [128, 128], BF16, isOutput=False)
    y = nc.declare_dram_parameter("y", [NLOC, DIM], F32, isOutput=True)

    with tile.TileContext(nc) as tc:
        with (
            tc.tile_pool(name="pw", bufs=1) as pw,      # persistent across phases
            tc.tile_pool(name="psA", bufs=3, space="PSUM") as psA,  # [128,1024] mixes
            tc.tile_pool(name="psB", bufs=2, space="PSUM") as psB,  # [128,512] scores/proj/@V
        ):
            # persistent small tensors
            wo_sb = [pw.tile([128, DIM], BF16, name=f"wo{i}", tag=f"wo{i}") for i in range(4)]
            th1s_sb = pw.tile([128, 32], F32, name="th1s", tag="th1s")
            t2_sb = pw.tile([128, 128], BF16, name="t2", tag="t2")
            bia_sb = pw.tile([1, 4 * DIM], BF16, name="bia", tag="bia")
            eye_sb = pw.tile([128, 128], BF16, name="eye", tag="eye")
            ones_sb = pw.tile([1, DIM], BF16, name="ones", tag="ones")
            nc.any.memset(ones_sb[:], 1.0)

            # persistent activations: th1-fused interleaved Qhat, K^T, V
            # qhat[rc][p=(h,d) in chunk rc, col (j, n16, g)] = th1[g,h] * Q^T[(h,d), n]
            #   with n = 16*j + n16 (local query index)
            # qhat[rc][p, j*128 + g*16 + n16] = th1[g, h(p)] * Q^T[(h,d), n]
            # with n = 16*j + n16: interleaved tiles use (g, n16) partition
            # order so the build writes dense 16-element runs
            qhat_sb = [pw.tile([128, NLOC * 8], BF16, name=f"qh{i}", tag=f"qh{i}")
                       for i in range(4)]
            kt_sb = [pw.tile([128, N], BF16, name=f"kt{i}", tag=f"kt{i}") for i in range(4)]
            v_sb = pw.tile([128, MT, DIM], BF16, name="v", tag="v")

            def evict(dst, src, use_act):
                if use_act:
                    nc.scalar.copy(dst, src)
                else:
                    nc.vector.tensor_copy(dst, src)

            # ================= phase A: x^T + QKV projection =================
            with tc.tile_pool(name="pxt", bufs=1) as pxt:
                qt_sb = [pxt.tile([128, NLOC], BF16, name=f"qt{i}", tag=f"qt{i}")
                         for i in range(4)]
                wq_sb = [pxt.tile([128, DIM], BF16, name=f"wq{i}", tag=f"wq{i}") for i in range(4)]
                wk_sb = [pxt.tile([128, DIM], BF16, name=f"wk{i}", tag=f"wk{i}") for i in range(4)]
                wv_sb = [pxt.tile([128, DIM], BF16, name=f"wv{i}", tag=f"wv{i}") for i in range(4)]
                # All HBM loads go through the gpsimd SWDGE queue in
                # need-order: HWDGE-queue DMAs serialize at ~2.5us each, and
                # xbar transposes flood all 16 DMA engines (~1.3us each), so
                # phase A avoids the xbar entirely: x^T is built with PE
                # identity-matmul transposes (the PE is idle here anyway).
                xr = x[:].rearrange("(c p) d -> p c d", p=128)
                sizes = [4, 4, 4, 4]
                offs = [0, 4, 8, 12]
                xbs = []          # per-tile (chunk tile, index within chunk)
                for ci, (sz, o) in enumerate(zip(sizes, offs)):
                    xb = pxt.tile([128, sz, DIM], BF16, name=f"xb{sz}",
                                  tag=f"xb{sz}", bufs=4)
                    nc.gpsimd.dma_start(xb[:], xr[:, o:o + sz, :])
                    xbs += [(xb, i) for i in range(sz)]
                    if ci == 0:
                        for i in range(4):
                            nc.gpsimd.dma_start(wk_sb[i][:], wk[128 * i:128 * (i + 1), :])
                        nc.gpsimd.dma_start(bia_sb[:], bia[:])
                        nc.gpsimd.dma_start(eye_sb[:], eye[:])
                        nc.gpsimd.dma_start(th1s_sb[:], th1s[:])
                        nc.gpsimd.dma_start(t2_sb[:], t2t[:])
                for i in range(4):
                    nc.gpsimd.dma_start(wq_sb[i][:], wq[128 * i:128 * (i + 1), :])
                    nc.gpsimd.dma_start(wv_sb[i][:], wv[128 * i:128 * (i + 1), :])
                for i in range(4):
                    nc.gpsimd.dma_start(wo_sb[i][:], wo[128 * i:128 * (i + 1), :])

                # xt_sb[p, t, j, f]: dim = j*128+p, key row m = t*128+f
                # query rows are tiles 0..NT-1 (host rolls them first)
                xt_sb = pxt.tile([128, MT, 4, 128], BF16, name="xt", tag="xt")
                ei = 0

                def tpose2(c2):
                    # PE-transpose two x tiles into one psB buffer (bf16
                    # bitcast view), evict as one [128,1024] copy
                    ps = psB.tile([128, 512], F32, name="psq", tag="pss")
                    pb = ps[:].bitcast(BF16)
                    for s in range(2):
                        xb, i = xbs[2 * c2 + s]
                        for j in range(4):
                            nc.tensor.transpose(
                                pb[:, 512 * s + 128 * j:512 * s + 128 * (j + 1)],
                                xb[:, i, 128 * j:128 * (j + 1)], eye_sb[:])
                    nc.vector.tensor_copy(
                        xt_sb[:, 2 * c2:2 * c2 + 2, :, :], pb)

                def kproj(mch):
                    nonlocal ei
                    for rc in range(4):
                        ps = psB.tile([128, 512], F32, name="psq", tag="pss")
                        for j in range(4):
                            rhs = xt_sb[:, 4 * mch:4 * (mch + 1), j, :]
                            nc.tensor.matmul(ps[:], wk_sb[j][:, 128 * rc:128 * (rc + 1)],
                                             rhs, start=(j == 0), stop=False)
                        nc.tensor.matmul(ps[:], bia_sb[0:1, 1 * DIM + 128 * rc:1 * DIM + 128 * (rc + 1)],
                                         ones_sb[:, 0:512], start=False, stop=True)
                        evict(kt_sb[rc][:, 512 * mch:512 * (mch + 1)], ps[:], ei % 2 == 0)
                        ei += 1

                for mch in range(N // 512):    # K^T over all keys (needed first)
                    tpose2(2 * mch)
                    tpose2(2 * mch + 1)
                    kproj(mch)
                for rc in range(4):            # Q^T own half, Qhat build interleaved
                    for nch in range(NLOC // 512):
                        ps = psB.tile([128, 512], F32, name="psq", tag="pss")
                        for j in range(4):
                            rhs = xt_sb[:, 4 * nch:4 * (nch + 1), j, :]
                            nc.tensor.matmul(ps[:], wq_sb[j][:, 128 * rc:128 * (rc + 1)],
                                             rhs, start=(j == 0), stop=False)
                        nc.tensor.matmul(ps[:], bia_sb[0:1, 0 * DIM + 128 * rc:0 * DIM + 128 * (rc + 1)],
                                         ones_sb[:, 0:512], start=False, stop=True)
                        evict(qt_sb[rc][:, 512 * nch:512 * (nch + 1)], ps[:], ei % 2 == 0)
                        ei += 1
                    qhr = qhat_sb[rc][:].rearrange("p (j g n) -> p j g n",
                                                   g=8, n=NG)
                    qtr = qt_sb[rc][:].rearrange("p (j n) -> p j n", n=NG)
                    for g in range(8):
                        nc.vector.tensor_scalar_mul(
                            qhr[:, :, g, :], qtr,
                            th1s_sb[:, 8 * rc + g:8 * rc + g + 1])
                def vproj(mt):
                    ps = psB.tile([128, 512], F32, name="psq", tag="pss")
                    for j in range(4):
                        nc.tensor.matmul(ps[:], xt_sb[:, mt, j, :], wv_sb[j][:],
                                         start=(j == 0), stop=False)
                    nc.tensor.matmul(ps[:], ones_sb[:, 0:128],
                                     bia_sb[0:1, 2 * DIM:3 * DIM],
                                     start=False, stop=True)
                    evict(v_sb[:, mt, :], ps[:], mt % 2 == 0)
                for mt in range(MT):
                    vproj(mt)
                vfill = []

            # ================= phase B: attention main loop =================
            # Fused design: th1 mix is folded into the score matmuls (Qhat
            # columns are th1-scaled, head-interleaved), so scores land
            # directly in interleaved PSUM: no raw-score evict, no pack DMA.
            # Pipeline: iteration i emits fscores+exp(i) woven with mix2(i-?)
            # and attn@V(i-1): PE bubbles while exp drains are filled by @V.
            with tc.tile_pool(name="pk", bufs=1) as pk:
                st = {}

                def mk_fscore(t):
                    # at tiles are allocated by mk_m2 (their first writer, one
                    # pipeline slot later) to keep ring pressure low at bufs=3
                    st[t] = {"u": {}, "zz": {}}

                    def fscore(j):
                        u = pk.tile([128, N], BF16, name="u", tag="u", bufs=3)
                        zz = pk.tile([128, 68], F32, name="zz", tag="zz", bufs=3)
                        st[t]["u"][j] = u
                        st[t]["zz"][j] = zz
                        for half in range(2):
                            ps = psA.tile([128, 1024], F32, name="psm1", tag="psm")
                            for mc in range(2):
                                m0 = 1024 * half + 512 * mc
                                for rc in range(4):
                                    nc.tensor.matmul(
                                        ps[:, 512 * mc:512 * (mc + 1)],
                                        qhat_sb[rc][:, 128 * (8 * t + j):
                                                    128 * (8 * t + j) + 128],
                                        kt_sb[rc][:, m0:m0 + 512],
                                        start=(rc == 0), stop=(rc == 3))
                            nc.scalar.activation(u[:, 1024 * half:1024 * (half + 1)],
                                                 ps[:], AF.Exp,
                                                 accum_out=zz[:, half:half + 1])
                        # l2 build hoisted here: ready well before mix2 needs
                        # it so the mix2 matmuls never stall the PE queue.
                        nc.vector.tensor_add(zz[:, 2:3], zz[:, 0:1], zz[:, 1:2])
                        nc.vector.reciprocal(zz[:, 3:4], zz[:, 2:3])
                        l2 = zz[:, 4:68].bitcast(BF16)
                        nc.vector.tensor_scalar_mul(l2, t2_sb[:], zz[:, 3:4])
                    return fscore

                def mk_m2(t):
                    st[t]["at"] = [pk.tile([128, 8, 8, 128], BF16, name=f"at{i}",
                                           tag="at", bufs=4) for i in range(2)]
                    at_h = st[t]["at"]

                    def m2(j):
                        u = st[t]["u"][j]
                        zz = st[t]["zz"][j]
                        l2 = zz[:, 4:68].bitcast(BF16)
                        a = pk.tile([128, N], BF16, name="a", tag="a", bufs=3)
                        for half in range(2):
                            ps = psA.tile([128, 1024], F32, name="psm2", tag="psm")
                            for mc in range(2):
                                m0 = 1024 * half + 512 * mc
                                nc.tensor.matmul(ps[:, 512 * mc:512 * (mc + 1)],
                                                 l2, u[:, m0:m0 + 512],
                                                 start=True, stop=True)
                            evict(a[:, 1024 * half:1024 * (half + 1)], ps[:],
                                  False)
                        for half in range(2):
                            nc.sync.dma_start_transpose(
                                at_h[half][:, j, :, :],
                                a[:, 1024 * half:1024 * (half + 1)])
                    return m2

                def mk_av(tv):
                    at_h = st[tv]["at"]
                    otb = pk.tile([128, 4, 128], BF16, name="otb", tag="otb", bufs=2)
                    st[tv]["otb"] = otb

                    def av(rc):
                        # the two heads of the pair run in the two 64-col
                        # halves of the PE array concurrently (col tiling)
                        ps = psB.tile([128, 128], F32, name="psv", tag="pss")
                        for half in range(2):
                            for mc in range(8):
                                mchunk = half * 8 + mc
                                for gi in range(2):
                                    g = 2 * rc + gi
                                    rhs = at_h[half][:, :, mc,
                                                     NG * g:NG * (g + 1)]
                                    nc.tensor.matmul(
                                        ps[64 * gi:64 * (gi + 1), :],
                                        v_sb[:, mchunk, 64 * g:64 * (g + 1)],
                                        rhs, start=(mchunk == 0),
                                        stop=(mchunk == 15),
                                        tile_position=(0, 64 * gi))
                        nc.vector.tensor_copy(otb[:, rc, :], ps[:])
                    return av

                def emit_outproj(tv):
                    otb = st[tv]["otb"]
                    ps = psB.tile([128, DIM], F32, name="pso", tag="pss")
                    for rc in range(4):
                        nc.tensor.matmul(ps[:], otb[:, rc, :], wo_sb[rc][:],
                                         start=(rc == 0), stop=False)
                    nc.tensor.matmul(ps[:], ones_sb[:, 0:128],
                                     bia_sb[0:1, 3 * DIM:4 * DIM],
                                     start=False, stop=True)
                    yt = pk.tile([128, DIM], F32, name="yt", tag="yt", bufs=2)
                    nc.vector.tensor_copy(yt[:], ps[:])
                    nc.gpsimd.dma_start(y[128 * tv:128 * (tv + 1), :], yt[:])
                    del st[tv]

                # Rotated software pipeline: the fscore stream runs two
                # 16-query groups ahead of m2, crossing tile boundaries, so
                # the in-order PE queue never drains behind a mix2 matmul
                # waiting on PSUM recycling at the end of a tile.
                fsc = {0: mk_fscore(0)}
                fsc[0](0)
                fsc[0](1)
                for i in range(NT + 1):
                    tm = i if i < NT else None
                    tv = i - 1 if i >= 1 else None
                    av = mk_av(tv) if tv is not None else None
                    avq = [0, 1, 2, 3] if tv is not None else []
                    if tm is None:           # last iteration: drain
                        while avq:
                            av(avq.pop(0))
                        emit_outproj(tv)
                        continue
                    m2 = mk_m2(tm)
                    for k in range(8):
                        m2(k)
                        ft, fj = (tm, k + 2) if k < 6 else (tm + 1, k - 6)
                        if ft < NT:
                            if fj == 0:
                                fsc[ft] = mk_fscore(ft)
                                fsc.pop(ft - 1, None)
                            fsc[ft](fj)
                        if avq and k % 2 == 1:
                            av(avq.pop(0))
                        elif vfill and tv is None:
                            for _ in range(3 if k % 2 == 1 else 2):
                                if vfill:
                                    vfill.pop(0)()
                    while avq:
                        av(avq.pop(0))
                    if tv is not None:
                        emit_outproj(tv)

    nc.compile()
    return nc


_NC_CACHE = None


def _get_nc():
    global _NC_CACHE
    if _NC_CACHE is None:
        _NC_CACHE = build_nc()
    return _NC_CACHE


def _host_prep(w_qkv, b_qkv, th1, th2, w_out, b_out):
    bf = ml_dtypes.bfloat16
    scale = DH ** -0.5
    w_qkv = np.asarray(w_qkv, dtype=np.float32)
    wq = (w_qkv[:, 0:DIM] * scale).astype(bf)
    wk = w_qkv[:, DIM:2 * DIM].astype(bf)
    wv = w_qkv[:, 2 * DIM:3 * DIM].astype(bf)
    wo = np.asarray(w_out, dtype=np.float32).astype(bf)
    th1 = np.asarray(th1, dtype=np.float32)
    th2 = np.asarray(th2, dtype=np.float32)
    # th1 spread for fused scores: th1s[p, rc*8+g] = th1[g, rc*2 + p//64]
    th1s = np.zeros((128, 32), dtype=np.float32)
    for rc in range(4):
        for g in range(8):
            for p in range(128):
                th1s[p, 8 * rc + g] = th1[g, rc * 2 + p // 64]
    # mix2 template, (g, n16) partition order:
    # t2t[g*16+n16, g2*16+n16] = th2[g2, g]
    t2t = np.zeros((128, 128), dtype=np.float32)
    for n16 in range(NG):
        t2t[n16::NG, n16::NG] = th2.T
    bqkv = np.asarray(b_qkv, dtype=np.float32)
    bia = np.zeros((1, 4 * DIM), dtype=np.float32)
    bia[0, 0:DIM] = bqkv[0:DIM] * scale     # q bias scaled with w_q
    bia[0, DIM:3 * DIM] = bqkv[DIM:3 * DIM]
    bia[0, 3 * DIM:] = np.asarray(b_out, dtype=np.float32)
    return (wq, wk, wv, wo, th1s, t2t.astype(bf), bia.astype(bf))


def _core_inputs(x, prepped):
    """Per-core input maps.  x is the full [B, N, DIM] fp32 array."""
    bf = ml_dtypes.bfloat16
    wq, wk, wv, wo, th1s, t2t, bia = prepped
    xbf = np.asarray(x, dtype=np.float32).astype(bf)
    in_maps = []
    for c in range(NCORES):
        b, half = c // 2, c % 2
        # roll so this core's query rows come first; key order is a
        # permutation shared by K and V, which attention is invariant to.
        xc = np.ascontiguousarray(
            np.roll(xbf[b], -NLOC * half, axis=0))
        in_maps.append({
            "x": xc,
            "wq": wq, "wk": wk, "wv": wv, "wo": wo,
            "th1s": th1s, "t2t": t2t, "bia": bia,
            "eye": np.eye(128, dtype=np.float32).astype(bf),
        })
    return in_maps


def kernel(x, w_qkv, b_qkv, th1, th2, w_out, b_out):
    prepped = _host_prep(w_qkv, b_qkv, th1, th2, w_out, b_out)
    nc = _get_nc()
    in_maps = _core_inputs(x, prepped)
    res = run_bass_kernel_spmd(nc, in_maps, core_ids=list(range(NCORES)))
    out = np.empty((B, N, DIM), dtype=np.float32)
    for c in range(NCORES):
        b, half = c // 2, c % 2
        out[b, NLOC * half:NLOC * (half + 1), :] = res.results[c]["y"]
    return out


# revision 29
# speedup vs baseline: 1.2244x; 1.2244x over previous
"""Talking-heads attention kernel for Trainium2, 8 NeuronCores.

Problem: B=4, N=2048, DIM=512, H=8, DH=64 talking-heads attention
(qkv proj -> per-head scores -> th1 head-mix -> softmax -> th2 head-mix
 -> attn @ v -> out proj).

Sharding: data-parallel over (batch, query-half): core c handles batch c//2,
query rows [1024*(c%2), 1024*(c%2)+1024).  Communication-free.  The host
rolls each core's copy of x so its own query rows come first; key order is
a permutation, which attention is invariant to (K and V share it).

Device pipeline per core (all matmuls bf16 with fp32 PSUM accumulation):
  1. x arrives bf16; PE identity-matmul transposes build x^T (the xbar is
     avoided in phase A: its descriptor flood serializes with the HBM
     loads); QKV projection on TensorE producing Q^T/K^T and V (n-major).
  2. Per 16-query group j: th1-fused scores land directly in
     head-interleaved PSUM [(g,n16), m] (qhat columns are th1-scaled).
  3. exp on ScalarE with fused per-row accumulation; the l2 stationary
     (th2[g,h]/Z row scaling) is built immediately after the exps so the
     mix2 matmuls never head-of-line-block the PE queue.
  4. mix2 output is xbar-DMA-transposed to key-major A^T, attn@V runs with
     V as the stationary operand, followed by the output projection.
"""

import sys

sys.path.insert(0, "/opt/trn_rl_repo")

import numpy as np
import ml_dtypes

import concourse.bass as bass
from concourse import bacc
import concourse.mybir as mybir
import concourse.tile as tile
from concourse.bass_utils import run_bass_kernel_spmd

BF16 = mybir.dt.bfloat16
F32 = mybir.dt.float32
AF = mybir.ActivationFunctionType

B, N, DIM = 4, 2048, 512
H, DH = 8, 64
NCORES = 8
NLOC = N // 2          # query rows per core
NT = NLOC // 128       # 8 query tiles per core
MT = N // 128          # 16 key chunks
NG = 16                # n16 group size in interleaved tiles


def build_nc():
    nc = bacc.Bacc()

    x = nc.declare_dram_parameter("x", [N, DIM], BF16, isOutput=False)
    wq = nc.declare_dram_parameter("wq", [DIM, DIM], BF16, isOutput=False)
    wk = nc.declare_dram_parameter("wk", [DIM, DIM], BF16, isOutput=False)
    wv = nc.declare_dram_parameter("wv", [DIM, DIM], BF16, isOutput=False)
    wo = nc.declare_dram_parameter("wo", [DIM, DIM], BF16, isOutput=False)
    t1t = nc.declare_dram_parameter("t1t", [128, 128], BF16, isOutput=False)
    t2t = nc.declare_dram_parameter("t2t", [128, 128], BF16, isOutput=False)
    # rows: bq (pre-scaled), bk, bv, bo
    bia = nc.declare_dram_parameter("bia", [1, 4 * DIM], BF16, isOutput=False)
    eye = nc.declare_dram_parameter("eye", [128, 128], BF16, isOutput=False)
    y = nc.declare_dram_parameter("y", [NLOC, DIM], F32, isOutput=True)

    with tile.TileContext(nc) as tc:
        with (
            tc.tile_pool(name="pw", bufs=1) as pw,      # persistent across phases
            tc.tile_pool(name="psA", bufs=3, space="PSUM") as psA,  # [128,1024] mixes
            tc.tile_pool(name="psB", bufs=2, space="PSUM") as psB,  # [128,512] scores/proj/@V
        ):
            # persistent small tensors
            wo_sb = [pw.tile([128, DIM], BF16, name=f"wo{i}", tag=f"wo{i}") for i in range(4)]
            t1_sb = pw.tile([128, 128], BF16, name="t1", tag="t1")
            t2_sb = pw.tile([128, 128], BF16, name="t2", tag="t2")
            bia_sb = pw.tile([1, 4 * DIM], BF16, name="bia", tag="bia")
            eye_sb = pw.tile([128, 128], BF16, name="eye", tag="eye")
            ones_sb = pw.tile([1, DIM], BF16, name="ones", tag="ones")
            nc.any.memset(ones_sb[:], 1.0)

            # persistent activations: block-masked Q^T (raw-score
            # stationaries), K^T, V.  qint[rc][p=(hrel,d), 32j+16*h2+n16] =
            # Q^T[(2rc+hrel,d), 16j+n16] if h2==hrel else 0: each 32-col
            # block feeds one of four concurrent col-tiled raw-score matmuls
            # producing S_h rows (16h+n16); th1 is applied afterwards as a
            # 128x128 mix matmul (t1t), halving score PE time vs the folded
            # 512-contraction form.
            qint_sb = [pw.tile([128, NLOC * 2], BF16, name=f"qi{i}", tag=f"qi{i}")
                       for i in range(4)]
            for i in range(4):
                nc.vector.memset(qint_sb[i][:], 0.0)
            kt_sb = [pw.tile([128, N], BF16, name=f"kt{i}", tag=f"kt{i}") for i in range(4)]
            v_sb = pw.tile([128, MT, DIM], BF16, name="v", tag="v")

            def evict(dst, src, use_act):
                if use_act:
                    nc.scalar.copy(dst, src)
                else:
                    nc.vector.tensor_copy(dst, src)

            # ================= phase A: x^T + QKV projection =================
            with tc.tile_pool(name="pxt", bufs=1) as pxt:
                wq_sb = [pxt.tile([128, DIM], BF16, name=f"wq{i}", tag=f"wq{i}") for i in range(4)]
                wk_sb = [pxt.tile([128, DIM], BF16, name=f"wk{i}", tag=f"wk{i}") for i in range(4)]
                wv_sb = [pxt.tile([128, DIM], BF16, name=f"wv{i}", tag=f"wv{i}") for i in range(4)]
                # All HBM loads go through the gpsimd SWDGE queue in
                # need-order: HWDGE-queue DMAs serialize at ~2.5us each, and
                # xbar transposes flood all 16 DMA engines (~1.3us each), so
                # phase A avoids the xbar entirely: x^T is built with PE
                # identity-matmul transposes (the PE is idle here anyway).
                xr = x[:].rearrange("(c p) d -> p c d", p=128)
                sizes = [4, 4, 4, 4]
                offs = [0, 4, 8, 12]
                xbs = []          # per-tile (chunk tile, index within chunk)
                for ci, (sz, o) in enumerate(zip(sizes, offs)):
                    xb = pxt.tile([128, sz, DIM], BF16, name=f"xb{sz}",
                                  tag=f"xb{sz}", bufs=4)
                    nc.gpsimd.dma_start(xb[:], xr[:, o:o + sz, :])
                    xbs += [(xb, i) for i in range(sz)]
                    if ci == 0:
                        for i in range(4):
                            nc.gpsimd.dma_start(wk_sb[i][:], wk[128 * i:128 * (i + 1), :])
                        nc.gpsimd.dma_start(bia_sb[:], bia[:])
                        nc.gpsimd.dma_start(eye_sb[:], eye[:])
                        nc.gpsimd.dma_start(t1_sb[:], t1t[:])
                        nc.gpsimd.dma_start(t2_sb[:], t2t[:])
                for i in range(4):
                    nc.gpsimd.dma_start(wq_sb[i][:], wq[128 * i:128 * (i + 1), :])
                    nc.gpsimd.dma_start(wv_sb[i][:], wv[128 * i:128 * (i + 1), :])
                for i in range(4):
                    nc.gpsimd.dma_start(wo_sb[i][:], wo[128 * i:128 * (i + 1), :])

                # xt_sb[p, t, j, f]: dim = j*128+p, key row m = t*128+f
                # query rows are tiles 0..NT-1 (host rolls them first)
                xt_sb = pxt.tile([128, MT, 4, 128], BF16, name="xt", tag="xt")
                ei = 0

                def tpose2(c2):
                    # PE-transpose two x tiles into one psB buffer (bf16
                    # bitcast view), evict as one [128,1024] copy
                    ps = psB.tile([128, 512], F32, name="psq", tag="pss")
                    pb = ps[:].bitcast(BF16)
                    for s in range(2):
                        xb, i = xbs[2 * c2 + s]
                        for j in range(4):
                            nc.tensor.transpose(
                                pb[:, 512 * s + 128 * j:512 * s + 128 * (j + 1)],
                                xb[:, i, 128 * j:128 * (j + 1)], eye_sb[:])
                    nc.vector.tensor_copy(
                        xt_sb[:, 2 * c2:2 * c2 + 2, :, :], pb)

                def kproj(mch):
                    nonlocal ei
                    for rc in range(4):
                        ps = psB.tile([128, 512], F32, name="psq", tag="pss")
                        for j in range(4):
                            rhs = xt_sb[:, 4 * mch:4 * (mch + 1), j, :]
                            nc.tensor.matmul(ps[:], wk_sb[j][:, 128 * rc:128 * (rc + 1)],
                                             rhs, start=(j == 0), stop=False)
                        nc.tensor.matmul(ps[:], bia_sb[0:1, 1 * DIM + 128 * rc:1 * DIM + 128 * (rc + 1)],
                                         ones_sb[:, 0:512], start=False, stop=True)
                        evict(kt_sb[rc][:, 512 * mch:512 * (mch + 1)], ps[:], ei % 2 == 0)
                        ei += 1

                for mch in range(N // 512):    # K^T over all keys (needed first)
                    tpose2(2 * mch)
                    tpose2(2 * mch + 1)
                    kproj(mch)
                for rc in range(4):            # Q^T own half -> masked qint
                    qr = qint_sb[rc][:].rearrange("p (j h2 n) -> p j h2 n",
                                                  h2=2, n=NG)
                    for nch in range(NLOC // 512):
                        ps = psB.tile([128, 512], F32, name="psq", tag="pss")
                        for j in range(4):
                            rhs = xt_sb[:, 4 * nch:4 * (nch + 1), j, :]
                            nc.tensor.matmul(ps[:], wq_sb[j][:, 128 * rc:128 * (rc + 1)],
                                             rhs, start=(j == 0), stop=False)
                        nc.tensor.matmul(ps[:], bia_sb[0:1, 0 * DIM + 128 * rc:0 * DIM + 128 * (rc + 1)],
                                         ones_sb[:, 0:512], start=False, stop=True)
                        psv = ps[:].rearrange("p (j n) -> p j n", n=NG)
                        evict(qr[0:64, 32 * nch:32 * (nch + 1), 0, :],
                              psv[0:64, :, :], ei % 2 == 0)
                        evict(qr[64:128, 32 * nch:32 * (nch + 1), 1, :],
                              psv[64:128, :, :], ei % 2 == 1)
                        ei += 1
                def vproj(mt):
                    ps = psB.tile([128, 512], F32, name="psq", tag="pss")
                    for j in range(4):
                        nc.tensor.matmul(ps[:], xt_sb[:, mt, j, :], wv_sb[j][:],
                                         start=(j == 0), stop=False)
                    nc.tensor.matmul(ps[:], ones_sb[:, 0:128],
                                     bia_sb[0:1, 2 * DIM:3 * DIM],
                                     start=False, stop=True)
                    evict(v_sb[:, mt, :], ps[:], mt % 2 == 0)
                for mt in range(MT):
                    vproj(mt)
                vfill = []

            # ================= phase B: attention main loop =================
            # Fused design: th1 mix is folded into the score matmuls (Qhat
            # columns are th1-scaled, head-interleaved), so scores land
            # directly in interleaved PSUM: no raw-score evict, no pack DMA.
            # Pipeline: iteration i emits fscores+exp(i) woven with mix2(i-?)
            # and attn@V(i-1): PE bubbles while exp drains are filled by @V.
            with tc.tile_pool(name="pk", bufs=1) as pk:
                st = {}

                def mk_fraw(t):
                    # at tiles are allocated by mk_m2 (their first writer) to
                    # keep ring pressure low
                    st[t] = {"u": {}, "zz": {}, "r": {}}

                    def fraw(j):
                        # raw per-head scores S_h rows (16h+n16): four
                        # col-tiled matmuls run concurrently in disjoint
                        # 32-col sections of the PE array
                        rr = []
                        for c in range(4):
                            ps = psB.tile([128, 512], F32, name="psr", tag="pss")
                            m0 = 512 * c
                            for rc in range(4):
                                nc.tensor.matmul(
                                    ps[32 * rc:32 * (rc + 1), :],
                                    qint_sb[rc][:, 32 * (8 * t + j):
                                                32 * (8 * t + j) + 32],
                                    kt_sb[rc][:, m0:m0 + 512],
                                    start=True, stop=True,
                                    tile_position=(0, 32 * rc))
                            r = pk.tile([128, 512], BF16, name="r", tag="r",
                                        bufs=8)
                            evict(r[:], ps[:], c % 2 == 0)
                            rr.append(r)
                        st[t]["r"][j] = rr
                    return fraw

                def mk_fmix(t):
                    def fmix(j):
                        u = pk.tile([128, N], BF16, name="u", tag="u", bufs=3)
                        zz = pk.tile([128, 68], F32, name="zz", tag="zz", bufs=3)
                        st[t]["u"][j] = u
                        st[t]["zz"][j] = zz
                        rr = st[t]["r"][j]
                        for half in range(2):
                            ps = psA.tile([128, 1024], F32, name="psm1", tag="psm")
                            for mc in range(2):
                                nc.tensor.matmul(ps[:, 512 * mc:512 * (mc + 1)],
                                                 t1_sb[:], rr[2 * half + mc][:],
                                                 start=True, stop=True)
                            nc.scalar.activation(u[:, 1024 * half:1024 * (half + 1)],
                                                 ps[:], AF.Exp,
                                                 accum_out=zz[:, half:half + 1])
                        # l2 build hoisted here: ready well before mix2 needs
                        # it so the mix2 matmuls never stall the PE queue.
                        nc.vector.tensor_add(zz[:, 2:3], zz[:, 0:1], zz[:, 1:2])
                        nc.vector.reciprocal(zz[:, 3:4], zz[:, 2:3])
                        l2 = zz[:, 4:68].bitcast(BF16)
                        nc.vector.tensor_scalar_mul(l2, t2_sb[:], zz[:, 3:4])
                    return fmix

                def mk_m2(t):
                    st[t]["at"] = [pk.tile([128, 8, 8, 128], BF16, name=f"at{i}",
                                           tag="at", bufs=4) for i in range(2)]
                    at_h = st[t]["at"]

                    def m2(j):
                        u = st[t]["u"][j]
                        zz = st[t]["zz"][j]
                        l2 = zz[:, 4:68].bitcast(BF16)
                        a = pk.tile([128, N], BF16, name="a", tag="a", bufs=3)
                        for half in range(2):
                            ps = psA.tile([128, 1024], F32, name="psm2", tag="psm")
                            for mc in range(2):
                                m0 = 1024 * half + 512 * mc
                                nc.tensor.matmul(ps[:, 512 * mc:512 * (mc + 1)],
                                                 l2, u[:, m0:m0 + 512],
                                                 start=True, stop=True)
                            evict(a[:, 1024 * half:1024 * (half + 1)], ps[:],
                                  False)
                        for half in range(2):
                            nc.sync.dma_start_transpose(
                                at_h[half][:, j, :, :],
                                a[:, 1024 * half:1024 * (half + 1)])
                    return m2

                def mk_av(tv):
                    at_h = st[tv]["at"]
                    otb = pk.tile([128, 4, 128], BF16, name="otb", tag="otb", bufs=2)
                    st[tv]["otb"] = otb

                    def av(rc):
                        # the two heads of the pair run in the two 64-col
                        # halves of the PE array concurrently (col tiling)
                        ps = psB.tile([128, 128], F32, name="psv", tag="pss")
                        for half in range(2):
                            for mc in range(8):
                                mchunk = half * 8 + mc
                                for gi in range(2):
                                    g = 2 * rc + gi
                                    rhs = at_h[half][:, :, mc,
                                                     NG * g:NG * (g + 1)]
                                    nc.tensor.matmul(
                                        ps[64 * gi:64 * (gi + 1), :],
                                        v_sb[:, mchunk, 64 * g:64 * (g + 1)],
                                        rhs, start=(mchunk == 0),
                                        stop=(mchunk == 15),
                                        tile_position=(0, 64 * gi))
                        nc.scalar.copy(otb[:, rc, :], ps[:])
                    return av

                def emit_outproj(tv):
                    otb = st[tv]["otb"]
                    ps = psB.tile([128, DIM], F32, name="pso", tag="pss")
                    for rc in range(4):
                        nc.tensor.matmul(ps[:], otb[:, rc, :], wo_sb[rc][:],
                                         start=(rc == 0), stop=False)
                    nc.tensor.matmul(ps[:], ones_sb[:, 0:128],
                                     bia_sb[0:1, 3 * DIM:4 * DIM],
                                     start=False, stop=True)
                    yt = pk.tile([128, DIM], F32, name="yt", tag="yt", bufs=2)
                    nc.scalar.copy(yt[:], ps[:])
                    nc.gpsimd.dma_start(y[128 * tv:128 * (tv + 1), :], yt[:])
                    del st[tv]

                # Rotated software pipeline: the fscore stream runs two
                # 16-query groups ahead of m2, crossing tile boundaries, so
                # the in-order PE queue never drains behind a mix2 matmul
                # waiting on PSUM recycling at the end of a tile.
                fr = {0: mk_fraw(0)}
                fm = {0: mk_fmix(0)}
                fr[0](0)
                fr[0](1)
                fm[0](0)
                for i in range(NT + 1):
                    tm = i if i < NT else None
                    tv = i - 1 if i >= 1 else None
                    av = mk_av(tv) if tv is not None else None
                    avq = [0, 1, 2, 3] if tv is not None else []
                    if tm is None:           # last iteration: drain
                        while avq:
                            av(avq.pop(0))
                        emit_outproj(tv)
                        continue
                    m2 = mk_m2(tm)
                    for k in range(8):
                        rt, rj = (tm, k + 2) if k < 6 else (tm + 1, k - 6)
                        if rt < NT:
                            if rj == 0:
                                fr[rt] = mk_fraw(rt)
                                fr.pop(rt - 1, None)
                            fr[rt](rj)
                        mt_, mj = (tm, k + 1) if k < 7 else (tm + 1, 0)
                        if mt_ < NT:
                            if mj == 0:
                                fm[mt_] = mk_fmix(mt_)
                                fm.pop(mt_ - 1, None)
                            fm[mt_](mj)
                        m2(k)
                        if avq and k % 2 == 1:
                            av(avq.pop(0))
                        elif vfill and tv is None:
                            for _ in range(3 if k % 2 == 1 else 2):
                                if vfill:
                                    vfill.pop(0)()
                    while avq:
                        av(avq.pop(0))
                    if tv is not None:
                        emit_outproj(tv)

    nc.compile()
    return nc


_NC_CACHE = None


def _get_nc():
    global _NC_CACHE
    if _NC_CACHE is None:
        _NC_CACHE = build_nc()
    return _NC_CACHE


def _host_prep(w_qkv, b_qkv, th1, th2, w_out, b_out):
    bf = ml_dtypes.bfloat16
    scale = DH ** -0.5
    w_qkv = np.asarray(w_qkv, dtype=np.float32)
    wq = (w_qkv[:, 0:DIM] * scale).astype(bf)
    wk = w_qkv[:, DIM:2 * DIM].astype(bf)
    wv = w_qkv[:, 2 * DIM:3 * DIM].astype(bf)
    wo = np.asarray(w_out, dtype=np.float32).astype(bf)
    th1 = np.asarray(th1, dtype=np.float32)
    th2 = np.asarray(th2, dtype=np.float32)
    # mix1 template, rows (h,n16)=16h+n16, cols (g,n16):
    # t1t[h*16+n16, g*16+n16] = th1[g, h]
    t1t = np.zeros((128, 128), dtype=np.float32)
    for n16 in range(NG):
        t1t[n16::NG, n16::NG] = th1.T
    # mix2 template, (g, n16) partition order:
    # t2t[g*16+n16, g2*16+n16] = th2[g2, g]
    t2t = np.zeros((128, 128), dtype=np.float32)
    for n16 in range(NG):
        t2t[n16::NG, n16::NG] = th2.T
    bqkv = np.asarray(b_qkv, dtype=np.float32)
    bia = np.zeros((1, 4 * DIM), dtype=np.float32)
    bia[0, 0:DIM] = bqkv[0:DIM] * scale     # q bias scaled with w_q
    bia[0, DIM:3 * DIM] = bqkv[DIM:3 * DIM]
    bia[0, 3 * DIM:] = np.asarray(b_out, dtype=np.float32)
    return (wq, wk, wv, wo, t1t.astype(bf), t2t.astype(bf), bia.astype(bf))


def _core_inputs(x, prepped):
    """Per-core input maps.  x is the full [B, N, DIM] fp32 array."""
    bf = ml_dtypes.bfloat16
    wq, wk, wv, wo, t1t, t2t, bia = prepped
    xbf = np.asarray(x, dtype=np.float32).astype(bf)
    in_maps = []
    for c in range(NCORES):
        b, half = c // 2, c % 2
        # roll so this core's query rows come first; key order is a
        # permutation shared by K and V, which attention is invariant to.
        xc = np.ascontiguousarray(
            np.roll(xbf[b], -NLOC * half, axis=0))
        in_maps.append({
            "x": xc,
            "wq": wq, "wk": wk, "wv": wv, "wo": wo,
            "t1t": t1t, "t2t": t2t, "bia": bia,
            "eye": np.eye(128, dtype=np.float32).astype(bf),
        })
    return in_maps


def kernel(x, w_qkv, b_qkv, th1, th2, w_out, b_out):
    prepped = _host_prep(w_qkv, b_qkv, th1, th2, w_out, b_out)
    nc = _get_nc()
    in_maps = _core_inputs(x, prepped)
    res = run_bass_kernel_spmd(nc, in_maps, core_ids=list(range(NCORES)))
    out = np.empty((B, N, DIM), dtype=np.float32)
    for c in range(NCORES):
        b, half = c // 2, c % 2
        out[b, NLOC * half:NLOC * (half + 1), :] = res.results[c]["y"]
    return out


# revision 30
# speedup vs baseline: 1.2770x; 1.0430x over previous
"""Talking-heads attention kernel for Trainium2, 8 NeuronCores.

Problem: B=4, N=2048, DIM=512, H=8, DH=64 talking-heads attention
(qkv proj -> per-head scores -> th1 head-mix -> softmax -> th2 head-mix
 -> attn @ v -> out proj).

Sharding: data-parallel over (batch, query-half): core c handles batch c//2,
query rows [1024*(c%2), 1024*(c%2)+1024).  Communication-free.  The host
rolls each core's copy of x so its own query rows come first; key order is
a permutation, which attention is invariant to (K and V share it).

Device pipeline per core (all matmuls bf16 with fp32 PSUM accumulation):
  1. x arrives bf16; PE identity-matmul transposes build x^T (the xbar is
     avoided in phase A: its descriptor flood serializes with the HBM
     loads); QKV projection on TensorE producing Q^T/K^T and V (n-major).
  2. Per 16-query group j: th1-fused scores land directly in
     head-interleaved PSUM [(g,n16), m] (qhat columns are th1-scaled).
  3. exp on ScalarE with fused per-row accumulation; the l2 stationary
     (th2[g,h]/Z row scaling) is built immediately after the exps so the
     mix2 matmuls never head-of-line-block the PE queue.
  4. mix2 output is xbar-DMA-transposed to key-major A^T, attn@V runs with
     V as the stationary operand, followed by the output projection.
"""

import sys

sys.path.insert(0, "/opt/trn_rl_repo")

import numpy as np
import ml_dtypes

import concourse.bass as bass
from concourse import bacc
import concourse.mybir as mybir
import concourse.tile as tile
from concourse.bass_utils import run_bass_kernel_spmd

BF16 = mybir.dt.bfloat16
F32 = mybir.dt.float32
AF = mybir.ActivationFunctionType

B, N, DIM = 4, 2048, 512
H, DH = 8, 64
NCORES = 8
NLOC = N // 2          # query rows per core
NT = NLOC // 128       # 8 query tiles per core
MT = N // 128          # 16 key chunks
NG = 16                # n16 group size in interleaved tiles


def build_nc():
    nc = bacc.Bacc()

    x = nc.declare_dram_parameter("x", [N, DIM], BF16, isOutput=False)
    wq = nc.declare_dram_parameter("wq", [DIM, DIM], BF16, isOutput=False)
    wk = nc.declare_dram_parameter("wk", [DIM, DIM], BF16, isOutput=False)
    wv = nc.declare_dram_parameter("wv", [DIM, DIM], BF16, isOutput=False)
    wo = nc.declare_dram_parameter("wo", [DIM, DIM], BF16, isOutput=False)
    t1t = nc.declare_dram_parameter("t1t", [128, 128], BF16, isOutput=False)
    t2t = nc.declare_dram_parameter("t2t", [128, 128], BF16, isOutput=False)
    # rows: bq (pre-scaled), bk, bv, bo
    bia = nc.declare_dram_parameter("bia", [1, 4 * DIM], BF16, isOutput=False)
    eye = nc.declare_dram_parameter("eye", [128, 128], BF16, isOutput=False)
    y = nc.declare_dram_parameter("y", [NLOC, DIM], F32, isOutput=True)

    with tile.TileContext(nc) as tc:
        with (
            tc.tile_pool(name="pw", bufs=1) as pw,      # persistent across phases
            tc.tile_pool(name="psA", bufs=3, space="PSUM") as psA,  # [128,1024] mixes
            tc.tile_pool(name="psB", bufs=2, space="PSUM") as psB,  # [128,512] scores/proj/@V
        ):
            # persistent small tensors
            wo_sb = [pw.tile([128, DIM], BF16, name=f"wo{i}", tag=f"wo{i}") for i in range(4)]
            t1_sb = pw.tile([128, 128], BF16, name="t1", tag="t1")
            t2_sb = pw.tile([128, 128], BF16, name="t2", tag="t2")
            bia_sb = pw.tile([1, 4 * DIM], BF16, name="bia", tag="bia")
            eye_sb = pw.tile([128, 128], BF16, name="eye", tag="eye")
            ones_sb = pw.tile([1, DIM], BF16, name="ones", tag="ones")
            nc.any.memset(ones_sb[:], 1.0)

            # persistent activations: block-masked Q^T (raw-score
            # stationaries), K^T, V.  qint[rc][p=(hrel,d), 32j+16*h2+n16] =
            # Q^T[(2rc+hrel,d), 16j+n16] if h2==hrel else 0: each 32-col
            # block feeds one of four concurrent col-tiled raw-score matmuls
            # producing S_h rows (16h+n16); th1 is applied afterwards as a
            # 128x128 mix matmul (t1t), halving score PE time vs the folded
            # 512-contraction form.
            qint_sb = [pw.tile([128, NLOC * 2], BF16, name=f"qi{i}", tag=f"qi{i}")
                       for i in range(4)]
            for i in range(4):
                nc.vector.memset(qint_sb[i][:], 0.0)
            kt_sb = [pw.tile([128, N], BF16, name=f"kt{i}", tag=f"kt{i}") for i in range(4)]
            v_sb = pw.tile([128, MT, DIM], BF16, name="v", tag="v")

            def evict(dst, src, use_act):
                if use_act:
                    nc.scalar.copy(dst, src)
                else:
                    nc.vector.tensor_copy(dst, src)

            # ================= phase A: x^T + QKV projection =================
            with tc.tile_pool(name="pxt", bufs=1) as pxt:
                wq_sb = [pxt.tile([128, DIM], BF16, name=f"wq{i}", tag=f"wq{i}") for i in range(4)]
                wk_sb = [pxt.tile([128, DIM], BF16, name=f"wk{i}", tag=f"wk{i}") for i in range(4)]
                wv_sb = [pxt.tile([128, DIM], BF16, name=f"wv{i}", tag=f"wv{i}") for i in range(4)]
                # All HBM loads go through the gpsimd SWDGE queue in
                # need-order: HWDGE-queue DMAs serialize at ~2.5us each, and
                # xbar transposes flood all 16 DMA engines (~1.3us each), so
                # phase A avoids the xbar entirely: x^T is built with PE
                # identity-matmul transposes (the PE is idle here anyway).
                xr = x[:].rearrange("(c p) d -> p c d", p=128)
                sizes = [4, 4, 4, 4]
                offs = [0, 4, 8, 12]
                xbs = []          # per-tile (chunk tile, index within chunk)
                for ci, (sz, o) in enumerate(zip(sizes, offs)):
                    xb = pxt.tile([128, sz, DIM], BF16, name=f"xb{sz}",
                                  tag=f"xb{sz}", bufs=4)
                    nc.gpsimd.dma_start(xb[:], xr[:, o:o + sz, :])
                    xbs += [(xb, i) for i in range(sz)]
                    if ci == 0:
                        for i in range(4):
                            nc.gpsimd.dma_start(wk_sb[i][:], wk[128 * i:128 * (i + 1), :])
                        nc.gpsimd.dma_start(bia_sb[:], bia[:])
                        nc.gpsimd.dma_start(eye_sb[:], eye[:])
                        nc.gpsimd.dma_start(t1_sb[:], t1t[:])
                        nc.gpsimd.dma_start(t2_sb[:], t2t[:])
                for i in range(4):
                    nc.gpsimd.dma_start(wq_sb[i][:], wq[128 * i:128 * (i + 1), :])
                    nc.gpsimd.dma_start(wv_sb[i][:], wv[128 * i:128 * (i + 1), :])
                for i in range(4):
                    nc.gpsimd.dma_start(wo_sb[i][:], wo[128 * i:128 * (i + 1), :])

                # xt_sb[p, t, j, f]: dim = j*128+p, key row m = t*128+f
                # query rows are tiles 0..NT-1 (host rolls them first)
                xt_sb = pxt.tile([128, MT, 4, 128], BF16, name="xt", tag="xt")
                ei = 0

                def tpose2(c2):
                    # PE-transpose two x tiles into one psB buffer (bf16
                    # bitcast view), evict as one [128,1024] copy
                    ps = psB.tile([128, 512], F32, name="psq", tag="pss")
                    pb = ps[:].bitcast(BF16)
                    for s in range(2):
                        xb, i = xbs[2 * c2 + s]
                        for j in range(4):
                            nc.tensor.transpose(
                                pb[:, 512 * s + 128 * j:512 * s + 128 * (j + 1)],
                                xb[:, i, 128 * j:128 * (j + 1)], eye_sb[:])
                    nc.vector.tensor_copy(
                        xt_sb[:, 2 * c2:2 * c2 + 2, :, :], pb)

                def kproj(mch):
                    nonlocal ei
                    for rc in range(4):
                        ps = psB.tile([128, 512], F32, name="psq", tag="pss")
                        for j in range(4):
                            rhs = xt_sb[:, 4 * mch:4 * (mch + 1), j, :]
                            nc.tensor.matmul(ps[:], wk_sb[j][:, 128 * rc:128 * (rc + 1)],
                                             rhs, start=(j == 0), stop=False)
                        nc.tensor.matmul(ps[:], bia_sb[0:1, 1 * DIM + 128 * rc:1 * DIM + 128 * (rc + 1)],
                                         ones_sb[:, 0:512], start=False, stop=True)
                        evict(kt_sb[rc][:, 512 * mch:512 * (mch + 1)], ps[:], ei % 2 == 0)
                        ei += 1

                for mch in range(N // 512):    # K^T over all keys (needed first)
                    tpose2(2 * mch)
                    tpose2(2 * mch + 1)
                    kproj(mch)
                for rc in range(4):            # Q^T own half -> masked qint
                    qr = qint_sb[rc][:].rearrange("p (j h2 n) -> p j h2 n",
                                                  h2=2, n=NG)
                    for nch in range(NLOC // 512):
                        ps = psB.tile([128, 512], F32, name="psq", tag="pss")
                        for j in range(4):
                            rhs = xt_sb[:, 4 * nch:4 * (nch + 1), j, :]
                            nc.tensor.matmul(ps[:], wq_sb[j][:, 128 * rc:128 * (rc + 1)],
                                             rhs, start=(j == 0), stop=False)
                        nc.tensor.matmul(ps[:], bia_sb[0:1, 0 * DIM + 128 * rc:0 * DIM + 128 * (rc + 1)],
                                         ones_sb[:, 0:512], start=False, stop=True)
                        psv = ps[:].rearrange("p (j n) -> p j n", n=NG)
                        evict(qr[0:64, 32 * nch:32 * (nch + 1), 0, :],
                              psv[0:64, :, :], ei % 2 == 0)
                        evict(qr[64:128, 32 * nch:32 * (nch + 1), 1, :],
                              psv[64:128, :, :], ei % 2 == 1)
                        ei += 1
                def vproj(mt):
                    ps = psB.tile([128, 512], F32, name="psq", tag="pss")
                    for j in range(4):
                        nc.tensor.matmul(ps[:], xt_sb[:, mt, j, :], wv_sb[j][:],
                                         start=(j == 0), stop=False)
                    nc.tensor.matmul(ps[:], ones_sb[:, 0:128],
                                     bia_sb[0:1, 2 * DIM:3 * DIM],
                                     start=False, stop=True)
                    evict(v_sb[:, mt, :], ps[:], mt % 2 == 0)
                for mt in range(MT):
                    vproj(mt)
                vfill = []

            # ================= phase B: attention main loop =================
            # Fused design: th1 mix is folded into the score matmuls (Qhat
            # columns are th1-scaled, head-interleaved), so scores land
            # directly in interleaved PSUM: no raw-score evict, no pack DMA.
            # Pipeline: iteration i emits fscores+exp(i) woven with mix2(i-?)
            # and attn@V(i-1): PE bubbles while exp drains are filled by @V.
            with tc.tile_pool(name="pk", bufs=1) as pk:
                st = {}

                def mk_fraw(t):
                    # at tiles are allocated by mk_m2 (their first writer) to
                    # keep ring pressure low
                    st[t] = {"u": {}, "zz": {}, "r": {}}

                    def fraw_chunk(j, c):
                        # raw per-head scores S_h rows (16h+n16): four
                        # col-tiled matmuls run concurrently in disjoint
                        # 32-col sections of the PE array
                        ps = psB.tile([128, 512], F32, name="psr", tag="pss")
                        m0 = 512 * c
                        for rc in range(4):
                            nc.tensor.matmul(
                                ps[32 * rc:32 * (rc + 1), :],
                                qint_sb[rc][:, 32 * (8 * t + j):
                                            32 * (8 * t + j) + 32],
                                kt_sb[rc][:, m0:m0 + 512],
                                start=True, stop=True,
                                tile_position=(0, 32 * rc))
                        r = pk.tile([128, 512], BF16, name="r", tag="r",
                                    bufs=8)
                        evict(r[:], ps[:], c % 2 == 0)
                        st[t]["r"].setdefault(j, []).append(r)
                    return fraw_chunk

                def mk_fmix(t):
                    def fmix_half(j, half):
                        if half == 0:
                            st[t]["u"][j] = pk.tile([128, N], BF16, name="u",
                                                    tag="u", bufs=3)
                            st[t]["zz"][j] = pk.tile([128, 68], F32, name="zz",
                                                     tag="zz", bufs=3)
                        u = st[t]["u"][j]
                        zz = st[t]["zz"][j]
                        rr = st[t]["r"][j]
                        ps = psA.tile([128, 1024], F32, name="psm1", tag="psm")
                        for mc in range(2):
                            nc.tensor.matmul(ps[:, 512 * mc:512 * (mc + 1)],
                                             t1_sb[:], rr[2 * half + mc][:],
                                             start=True, stop=True)
                        nc.scalar.activation(u[:, 1024 * half:1024 * (half + 1)],
                                             ps[:], AF.Exp,
                                             accum_out=zz[:, half:half + 1])
                        if half == 1:
                            # l2 build: ready well before mix2 needs it
                            nc.vector.tensor_add(zz[:, 2:3], zz[:, 0:1], zz[:, 1:2])
                            nc.vector.reciprocal(zz[:, 3:4], zz[:, 2:3])
                            l2 = zz[:, 4:68].bitcast(BF16)
                            nc.vector.tensor_scalar_mul(l2, t2_sb[:], zz[:, 3:4])
                    return fmix_half

                def mk_m2(t):
                    st[t]["at"] = [pk.tile([128, 8, 8, 128], BF16, name=f"at{i}",
                                           tag="at", bufs=4) for i in range(2)]
                    at_h = st[t]["at"]

                    def m2(j):
                        a = pk.tile([128, N], BF16, name="a", tag="a", bufs=3)

                        def m2_half(half):
                            u = st[t]["u"][j]
                            zz = st[t]["zz"][j]
                            l2 = zz[:, 4:68].bitcast(BF16)
                            ps = psA.tile([128, 1024], F32, name="psm2", tag="psm")
                            for mc in range(2):
                                m0 = 1024 * half + 512 * mc
                                nc.tensor.matmul(ps[:, 512 * mc:512 * (mc + 1)],
                                                 l2, u[:, m0:m0 + 512],
                                                 start=True, stop=True)
                            evict(a[:, 1024 * half:1024 * (half + 1)], ps[:],
                                  False)
                            nc.sync.dma_start_transpose(
                                at_h[half][:, j, :, :],
                                a[:, 1024 * half:1024 * (half + 1)])
                        return m2_half
                    return m2

                def mk_av(tv):
                    at_h = st[tv]["at"]
                    otb = pk.tile([128, 4, 128], BF16, name="otb", tag="otb", bufs=2)
                    st[tv]["otb"] = otb

                    def av(rc):
                        # the two heads of the pair run in the two 64-col
                        # halves of the PE array concurrently (col tiling)
                        ps = psB.tile([128, 128], F32, name="psv", tag="pss")
                        for half in range(2):
                            for mc in range(8):
                                mchunk = half * 8 + mc
                                for gi in range(2):
                                    g = 2 * rc + gi
                                    rhs = at_h[half][:, :, mc,
                                                     NG * g:NG * (g + 1)]
                                    nc.tensor.matmul(
                                        ps[64 * gi:64 * (gi + 1), :],
                                        v_sb[:, mchunk, 64 * g:64 * (g + 1)],
                                        rhs, start=(mchunk == 0),
                                        stop=(mchunk == 15),
                                        tile_position=(0, 64 * gi))
                        nc.scalar.copy(otb[:, rc, :], ps[:])
                    return av

                def emit_outproj(tv):
                    otb = st[tv]["otb"]
                    ps = psB.tile([128, DIM], F32, name="pso", tag="pss")
                    for rc in range(4):
                        nc.tensor.matmul(ps[:], otb[:, rc, :], wo_sb[rc][:],
                                         start=(rc == 0), stop=False)
                    nc.tensor.matmul(ps[:], ones_sb[:, 0:128],
                                     bia_sb[0:1, 3 * DIM:4 * DIM],
                                     start=False, stop=True)
                    yt = pk.tile([128, DIM], F32, name="yt", tag="yt", bufs=2)
                    nc.scalar.copy(yt[:], ps[:])
                    nc.gpsimd.dma_start(y[128 * tv:128 * (tv + 1), :], yt[:])
                    del st[tv]

                # Rotated software pipeline: the fscore stream runs two
                # 16-query groups ahead of m2, crossing tile boundaries, so
                # the in-order PE queue never drains behind a mix2 matmul
                # waiting on PSUM recycling at the end of a tile.
                fr = {0: mk_fraw(0)}
                fm = {0: mk_fmix(0)}
                for c in range(4):
                    fr[0](0, c)
                for c in range(4):
                    fr[0](1, c)
                fm[0](0, 0)
                fm[0](0, 1)
                for i in range(NT + 1):
                    tm = i if i < NT else None
                    tv = i - 1 if i >= 1 else None
                    av = mk_av(tv) if tv is not None else None
                    avq = [0, 1, 2, 3] if tv is not None else []
                    if tm is None:           # last iteration: drain
                        while avq:
                            av(avq.pop(0))
                        emit_outproj(tv)
                        continue
                    m2 = mk_m2(tm)
                    for k in range(8):
                        rt, rj = (tm, k + 2) if k < 6 else (tm + 1, k - 6)
                        if rt < NT and rj == 0:
                            fr[rt] = mk_fraw(rt)
                            fr.pop(rt - 1, None)
                        mt_, mj = (tm, k + 1) if k < 7 else (tm + 1, 0)
                        if mt_ < NT and mj == 0:
                            fm[mt_] = mk_fmix(mt_)
                            fm.pop(mt_ - 1, None)
                        raw = (lambda c: fr[rt](rj, c)) if rt < NT else (lambda c: None)
                        mix = (lambda h: fm[mt_](mj, h)) if mt_ < NT else (lambda h: None)
                        # interleave so no two psB raw allocs are adjacent in
                        # the PE queue: the ring recycles behind other work
                        raw(0)
                        raw(1)
                        mix(0)
                        raw(2)
                        m2h = m2(k)
                        m2h(0)
                        raw(3)
                        mix(1)
                        m2h(1)
                        if avq and k % 2 == 1:
                            av(avq.pop(0))
                        elif vfill and tv is None:
                            for _ in range(3 if k % 2 == 1 else 2):
                                if vfill:
                                    vfill.pop(0)()
                    while avq:
                        av(avq.pop(0))
                    if tv is not None:
                        emit_outproj(tv)

    nc.compile()
    return nc


_NC_CACHE = None


def _get_nc():
    global _NC_CACHE
    if _NC_CACHE is None:
        _NC_CACHE = build_nc()
    return _NC_CACHE


def _host_prep(w_qkv, b_qkv, th1, th2, w_out, b_out):
    bf = ml_dtypes.bfloat16
    scale = DH ** -0.5
    w_qkv = np.asarray(w_qkv, dtype=np.float32)
    wq = (w_qkv[:, 0:DIM] * scale).astype(bf)
    wk = w_qkv[:, DIM:2 * DIM].astype(bf)
    wv = w_qkv[:, 2 * DIM:3 * DIM].astype(bf)
    wo = np.asarray(w_out, dtype=np.float32).astype(bf)
    th1 = np.asarray(th1, dtype=np.float32)
    th2 = np.asarray(th2, dtype=np.float32)
    # mix1 template, rows (h,n16)=16h+n16, cols (g,n16):
    # t1t[h*16+n16, g*16+n16] = th1[g, h]
    t1t = np.zeros((128, 128), dtype=np.float32)
    for n16 in range(NG):
        t1t[n16::NG, n16::NG] = th1.T
    # mix2 template, (g, n16) partition order:
    # t2t[g*16+n16, g2*16+n16] = th2[g2, g]
    t2t = np.zeros((128, 128), dtype=np.float32)
    for n16 in range(NG):
        t2t[n16::NG, n16::NG] = th2.T
    bqkv = np.asarray(b_qkv, dtype=np.float32)
    bia = np.zeros((1, 4 * DIM), dtype=np.float32)
    bia[0, 0:DIM] = bqkv[0:DIM] * scale     # q bias scaled with w_q
    bia[0, DIM:3 * DIM] = bqkv[DIM:3 * DIM]
    bia[0, 3 * DIM:] = np.asarray(b_out, dtype=np.float32)
    return (wq, wk, wv, wo, t1t.astype(bf), t2t.astype(bf), bia.astype(bf))


def _core_inputs(x, prepped):
    """Per-core input maps.  x is the full [B, N, DIM] fp32 array."""
    bf = ml_dtypes.bfloat16
    wq, wk, wv, wo, t1t, t2t, bia = prepped
    xbf = np.asarray(x, dtype=np.float32).astype(bf)
    in_maps = []
    for c in range(NCORES):
        b, half = c // 2, c % 2
        # roll so this core's query rows come first; key order is a
        # permutation shared by K and V, which attention is invariant to.
        xc = np.ascontiguousarray(
            np.roll(xbf[b], -NLOC * half, axis=0))
        in_maps.append({
            "x": xc,
            "wq": wq, "wk": wk, "wv": wv, "wo": wo,
            "t1t": t1t, "t2t": t2t, "bia": bia,
            "eye": np.eye(128, dtype=np.float32).astype(bf),
        })
    return in_maps


def kernel(x, w_qkv, b_qkv, th1, th2, w_out, b_out):
    prepped = _host_prep(w_qkv, b_qkv, th1, th2, w_out, b_out)
    nc = _get_nc()
    in_maps = _core_inputs(x, prepped)
    res = run_bass_kernel_spmd(nc, in_maps, core_ids=list(range(NCORES)))
    out = np.empty((B, N, DIM), dtype=np.float32)
    for c in range(NCORES):
        b, half = c // 2, c % 2
        out[b, NLOC * half:NLOC * (half + 1), :] = res.results[c]["y"]
    return out


# revision 33
# speedup vs baseline: 1.2882x; 1.0088x over previous
"""Talking-heads attention kernel for Trainium2, 8 NeuronCores.

Problem: B=4, N=2048, DIM=512, H=8, DH=64 talking-heads attention
(qkv proj -> per-head scores -> th1 head-mix -> softmax -> th2 head-mix
 -> attn @ v -> out proj).

Sharding: data-parallel over (batch, query-half): core c handles batch c//2,
query rows [1024*(c%2), 1024*(c%2)+1024).  Communication-free.  The host
rolls each core's copy of x so its own query rows come first; key order is
a permutation, which attention is invariant to (K and V share it).

Device pipeline per core (all matmuls bf16 with fp32 PSUM accumulation):
  1. x arrives bf16; PE identity-matmul transposes build x^T (the xbar is
     avoided in phase A: its descriptor flood serializes with the HBM
     loads); QKV projection on TensorE producing Q^T/K^T and V (n-major).
  2. Per 16-query group j: th1-fused scores land directly in
     head-interleaved PSUM [(g,n16), m] (qhat columns are th1-scaled).
  3. exp on ScalarE with fused per-row accumulation; the l2 stationary
     (th2[g,h]/Z row scaling) is built immediately after the exps so the
     mix2 matmuls never head-of-line-block the PE queue.
  4. mix2 output is xbar-DMA-transposed to key-major A^T, attn@V runs with
     V as the stationary operand, followed by the output projection.
"""

import sys

sys.path.insert(0, "/opt/trn_rl_repo")

import numpy as np
import ml_dtypes

import concourse.bass as bass
from concourse import bacc
import concourse.mybir as mybir
import concourse.tile as tile
from concourse.bass_utils import run_bass_kernel_spmd

BF16 = mybir.dt.bfloat16
F32 = mybir.dt.float32
AF = mybir.ActivationFunctionType

B, N, DIM = 4, 2048, 512
H, DH = 8, 64
NCORES = 8
NLOC = N // 2          # query rows per core
NT = NLOC // 128       # 8 query tiles per core
MT = N // 128          # 16 key chunks
NG = 16                # n16 group size in interleaved tiles


def build_nc():
    nc = bacc.Bacc()

    x = nc.declare_dram_parameter("x", [N, DIM], BF16, isOutput=False)
    wq = nc.declare_dram_parameter("wq", [DIM, DIM], BF16, isOutput=False)
    wk = nc.declare_dram_parameter("wk", [DIM, DIM], BF16, isOutput=False)
    wv = nc.declare_dram_parameter("wv", [DIM, DIM], BF16, isOutput=False)
    wo = nc.declare_dram_parameter("wo", [DIM, DIM], BF16, isOutput=False)
    t1t = nc.declare_dram_parameter("t1t", [128, 128], BF16, isOutput=False)
    t2t = nc.declare_dram_parameter("t2t", [128, 128], BF16, isOutput=False)
    # rows: bq (pre-scaled), bk, bv, bo
    bia = nc.declare_dram_parameter("bia", [1, 4 * DIM], BF16, isOutput=False)
    eye = nc.declare_dram_parameter("eye", [128, 128], BF16, isOutput=False)
    y = nc.declare_dram_parameter("y", [NLOC, DIM], F32, isOutput=True)

    with tile.TileContext(nc) as tc:
        with (
            tc.tile_pool(name="pw", bufs=1) as pw,      # persistent across phases
            tc.tile_pool(name="psA", bufs=3, space="PSUM") as psA,  # [128,1024] mixes
            tc.tile_pool(name="psB", bufs=2, space="PSUM") as psB,  # [128,512] scores/proj/@V
        ):
            # persistent small tensors
            wo_sb = [pw.tile([128, DIM], BF16, name=f"wo{i}", tag=f"wo{i}") for i in range(4)]
            t1_sb = pw.tile([128, 128], BF16, name="t1", tag="t1")
            t2_sb = pw.tile([128, 128], BF16, name="t2", tag="t2")
            bia_sb = pw.tile([1, 4 * DIM], BF16, name="bia", tag="bia")
            eye_sb = pw.tile([128, 128], BF16, name="eye", tag="eye")
            ones_sb = pw.tile([1, DIM], BF16, name="ones", tag="ones")
            nc.any.memset(ones_sb[:], 1.0)

            # persistent activations: block-masked Q^T (raw-score
            # stationaries), K^T, V.  qint[rc][p=(hrel,d), 32j+16*h2+n16] =
            # Q^T[(2rc+hrel,d), 16j+n16] if h2==hrel else 0: each 32-col
            # block feeds one of four concurrent col-tiled raw-score matmuls
            # producing S_h rows (16h+n16); th1 is applied afterwards as a
            # 128x128 mix matmul (t1t), halving score PE time vs the folded
            # 512-contraction form.
            qint_sb = [pw.tile([128, NLOC * 2], BF16, name=f"qi{i}", tag=f"qi{i}")
                       for i in range(4)]
            for i in range(4):
                nc.vector.memset(qint_sb[i][:], 0.0)
            kt_sb = [pw.tile([128, N], BF16, name=f"kt{i}", tag=f"kt{i}") for i in range(4)]
            v_sb = pw.tile([128, MT, DIM], BF16, name="v", tag="v")

            def evict(dst, src, use_act):
                if use_act:
                    nc.scalar.copy(dst, src)
                else:
                    nc.vector.tensor_copy(dst, src)

            # ================= phase A: x^T + QKV projection =================
            with tc.tile_pool(name="pxt", bufs=1) as pxt:
                wq_sb = [pxt.tile([128, DIM], BF16, name=f"wq{i}", tag=f"wq{i}") for i in range(4)]
                wk_sb = [pxt.tile([128, DIM], BF16, name=f"wk{i}", tag=f"wk{i}") for i in range(4)]
                wv_sb = [pxt.tile([128, DIM], BF16, name=f"wv{i}", tag=f"wv{i}") for i in range(4)]
                # All HBM loads go through the gpsimd SWDGE queue in
                # need-order: HWDGE-queue DMAs serialize at ~2.5us each, and
                # xbar transposes flood all 16 DMA engines (~1.3us each), so
                # phase A avoids the xbar entirely: x^T is built with PE
                # identity-matmul transposes (the PE is idle here anyway).
                xr = x[:].rearrange("(c p) d -> p c d", p=128)
                sizes = [4, 4, 4, 4]
                offs = [0, 4, 8, 12]
                xbs = []          # per-tile (chunk tile, index within chunk)
                for ci, (sz, o) in enumerate(zip(sizes, offs)):
                    xb = pxt.tile([128, sz, DIM], BF16, name=f"xb{sz}",
                                  tag=f"xb{sz}", bufs=4)
                    nc.gpsimd.dma_start(xb[:], xr[:, o:o + sz, :])
                    xbs += [(xb, i) for i in range(sz)]
                    if ci == 0:
                        for i in range(4):
                            nc.gpsimd.dma_start(wk_sb[i][:], wk[128 * i:128 * (i + 1), :])
                        nc.gpsimd.dma_start(bia_sb[:], bia[:])
                        nc.gpsimd.dma_start(eye_sb[:], eye[:])
                        nc.gpsimd.dma_start(t1_sb[:], t1t[:])
                        nc.gpsimd.dma_start(t2_sb[:], t2t[:])
                for i in range(4):
                    nc.gpsimd.dma_start(wq_sb[i][:], wq[128 * i:128 * (i + 1), :])
                    nc.gpsimd.dma_start(wv_sb[i][:], wv[128 * i:128 * (i + 1), :])
                for i in range(4):
                    nc.gpsimd.dma_start(wo_sb[i][:], wo[128 * i:128 * (i + 1), :])

                # xt_sb[p, t, j, f]: dim = j*128+p, key row m = t*128+f
                # query rows are tiles 0..NT-1 (host rolls them first)
                xt_sb = pxt.tile([128, MT, 4, 128], BF16, name="xt", tag="xt")
                ei = 0

                def tpose2(c2):
                    # PE-transpose two x tiles into one psB buffer (bf16
                    # bitcast view), evict as one [128,1024] copy
                    ps = psB.tile([128, 512], F32, name="psq", tag="pss")
                    pb = ps[:].bitcast(BF16)
                    for s in range(2):
                        xb, i = xbs[2 * c2 + s]
                        for j in range(4):
                            nc.tensor.transpose(
                                pb[:, 512 * s + 128 * j:512 * s + 128 * (j + 1)],
                                xb[:, i, 128 * j:128 * (j + 1)], eye_sb[:])
                    nc.vector.tensor_copy(
                        xt_sb[:, 2 * c2:2 * c2 + 2, :, :], pb)

                def kproj(mch):
                    nonlocal ei
                    for rc in range(4):
                        ps = psB.tile([128, 512], F32, name="psq", tag="pss")
                        for j in range(4):
                            rhs = xt_sb[:, 4 * mch:4 * (mch + 1), j, :]
                            nc.tensor.matmul(ps[:], wk_sb[j][:, 128 * rc:128 * (rc + 1)],
                                             rhs, start=(j == 0), stop=False)
                        nc.tensor.matmul(ps[:], bia_sb[0:1, 1 * DIM + 128 * rc:1 * DIM + 128 * (rc + 1)],
                                         ones_sb[:, 0:512], start=False, stop=True)
                        evict(kt_sb[rc][:, 512 * mch:512 * (mch + 1)], ps[:], ei % 2 == 0)
                        ei += 1

                for mch in range(N // 512):    # K^T over all keys (needed first)
                    tpose2(2 * mch)
                    tpose2(2 * mch + 1)
                    kproj(mch)
                for rc in range(4):            # Q^T own half -> masked qint
                    qr = qint_sb[rc][:].rearrange("p (j h2 n) -> p j h2 n",
                                                  h2=2, n=NG)
                    for nch in range(NLOC // 512):
                        ps = psB.tile([128, 512], F32, name="psq", tag="pss")
                        for j in range(4):
                            rhs = xt_sb[:, 4 * nch:4 * (nch + 1), j, :]
                            nc.tensor.matmul(ps[:], wq_sb[j][:, 128 * rc:128 * (rc + 1)],
                                             rhs, start=(j == 0), stop=False)
                        nc.tensor.matmul(ps[:], bia_sb[0:1, 0 * DIM + 128 * rc:0 * DIM + 128 * (rc + 1)],
                                         ones_sb[:, 0:512], start=False, stop=True)
                        psv = ps[:].rearrange("p (j n) -> p j n", n=NG)
                        evict(qr[0:64, 32 * nch:32 * (nch + 1), 0, :],
                              psv[0:64, :, :], ei % 2 == 0)
                        evict(qr[64:128, 32 * nch:32 * (nch + 1), 1, :],
                              psv[64:128, :, :], ei % 2 == 1)
                        ei += 1
                def vproj(mt):
                    ps = psB.tile([128, 512], F32, name="psq", tag="pss")
                    for j in range(4):
                        nc.tensor.matmul(ps[:], xt_sb[:, mt, j, :], wv_sb[j][:],
                                         start=(j == 0), stop=False)
                    nc.tensor.matmul(ps[:], ones_sb[:, 0:128],
                                     bia_sb[0:1, 2 * DIM:3 * DIM],
                                     start=False, stop=True)
                    evict(v_sb[:, mt, :], ps[:], mt % 2 == 0)
                for mt in range(MT):
                    vproj(mt)
                vfill = []

            # ================= phase B: attention main loop =================
            # Fused design: th1 mix is folded into the score matmuls (Qhat
            # columns are th1-scaled, head-interleaved), so scores land
            # directly in interleaved PSUM: no raw-score evict, no pack DMA.
            # Pipeline: iteration i emits fscores+exp(i) woven with mix2(i-?)
            # and attn@V(i-1): PE bubbles while exp drains are filled by @V.
            with tc.tile_pool(name="pk", bufs=1) as pk:
                st = {}

                def mk_fraw(t):
                    # at tiles are allocated by mk_m2 (their first writer) to
                    # keep ring pressure low
                    st[t] = {"u": {}, "zz": {}, "r": {}}

                    def fraw_chunk(j, c):
                        # raw per-head scores S_h rows (16h+n16): four
                        # col-tiled matmuls run concurrently in disjoint
                        # 32-col sections of the PE array
                        ps = psB.tile([128, 512], F32, name="psr", tag="pss")
                        m0 = 512 * c
                        for rc in range(4):
                            nc.tensor.matmul(
                                ps[32 * rc:32 * (rc + 1), :],
                                qint_sb[rc][:, 32 * (8 * t + j):
                                            32 * (8 * t + j) + 32],
                                kt_sb[rc][:, m0:m0 + 512],
                                start=True, stop=True,
                                tile_position=(0, 32 * rc))
                        r = pk.tile([128, 512], BF16, name="r", tag="r",
                                    bufs=8)
                        evict(r[:], ps[:], c % 2 == 0)
                        st[t]["r"].setdefault(j, []).append(r)
                    return fraw_chunk

                def mk_fmix(t):
                    def fmix_half(j, half):
                        if half == 0:
                            st[t]["u"][j] = pk.tile([128, N], BF16, name="u",
                                                    tag="u", bufs=3)
                            st[t]["zz"][j] = pk.tile([128, 68], F32, name="zz",
                                                     tag="zz", bufs=3)
                        u = st[t]["u"][j]
                        zz = st[t]["zz"][j]
                        rr = st[t]["r"][j]
                        ps = psA.tile([128, 1024], F32, name="psm1", tag="psm")
                        for mc in range(2):
                            nc.tensor.matmul(ps[:, 512 * mc:512 * (mc + 1)],
                                             t1_sb[:], rr[2 * half + mc][:],
                                             start=True, stop=True)
                        nc.scalar.activation(u[:, 1024 * half:1024 * (half + 1)],
                                             ps[:], AF.Exp,
                                             accum_out=zz[:, half:half + 1])
                        if half == 1:
                            # l2 build: ready well before mix2 needs it
                            nc.vector.tensor_add(zz[:, 2:3], zz[:, 0:1], zz[:, 1:2])
                            nc.vector.reciprocal(zz[:, 3:4], zz[:, 2:3])
                            l2 = zz[:, 4:68].bitcast(BF16)
                            nc.vector.tensor_scalar_mul(l2, t2_sb[:], zz[:, 3:4])
                    return fmix_half

                def mk_m2(t):
                    st[t]["at"] = [pk.tile([128, 8, 8, 128], BF16, name=f"at{i}",
                                           tag="at", bufs=4) for i in range(2)]
                    at_h = st[t]["at"]

                    def m2(j):
                        a = pk.tile([128, N], BF16, name="a", tag="a", bufs=3)

                        def m2_half(half):
                            u = st[t]["u"][j]
                            zz = st[t]["zz"][j]
                            l2 = zz[:, 4:68].bitcast(BF16)
                            ps = psA.tile([128, 1024], F32, name="psm2", tag="psm")
                            for mc in range(2):
                                m0 = 1024 * half + 512 * mc
                                nc.tensor.matmul(ps[:, 512 * mc:512 * (mc + 1)],
                                                 l2, u[:, m0:m0 + 512],
                                                 start=True, stop=True)
                            evict(a[:, 1024 * half:1024 * (half + 1)], ps[:],
                                  False)
                            nc.sync.dma_start_transpose(
                                at_h[half][:, j, :, :],
                                a[:, 1024 * half:1024 * (half + 1)])
                        return m2_half
                    return m2

                def mk_av(tv):
                    at_h = st[tv]["at"]
                    otb = pk.tile([128, 4, 128], BF16, name="otb", tag="otb", bufs=2)
                    st[tv]["otb"] = otb

                    def av(rc):
                        # the two heads of the pair run in the two 64-col
                        # halves of the PE array concurrently (col tiling)
                        ps = psB.tile([128, 128], F32, name="psv", tag="pss")
                        for half in range(2):
                            for mc in range(8):
                                mchunk = half * 8 + mc
                                for gi in range(2):
                                    g = 2 * rc + gi
                                    rhs = at_h[half][:, :, mc,
                                                     NG * g:NG * (g + 1)]
                                    nc.tensor.matmul(
                                        ps[64 * gi:64 * (gi + 1), :],
                                        v_sb[:, mchunk, 64 * g:64 * (g + 1)],
                                        rhs, start=(mchunk == 0),
                                        stop=(mchunk == 15),
                                        tile_position=(0, 64 * gi))
                        nc.scalar.copy(otb[:, rc, :], ps[:])
                    return av

                def emit_outproj(tv):
                    otb = st[tv]["otb"]
                    ps = psB.tile([128, DIM], F32, name="pso", tag="pss")
                    for rc in range(4):
                        nc.tensor.matmul(ps[:], otb[:, rc, :], wo_sb[rc][:],
                                         start=(rc == 0), stop=False)
                    nc.tensor.matmul(ps[:], ones_sb[:, 0:128],
                                     bia_sb[0:1, 3 * DIM:4 * DIM],
                                     start=False, stop=True)
                    yt = pk.tile([128, DIM], F32, name="yt", tag="yt", bufs=2)
                    nc.scalar.copy(yt[:], ps[:])
                    nc.gpsimd.dma_start(y[128 * tv:128 * (tv + 1), :], yt[:])
                    del st[tv]

                # Rotated software pipeline: the fscore stream runs two
                # 16-query groups ahead of m2, crossing tile boundaries, so
                # the in-order PE queue never drains behind a mix2 matmul
                # waiting on PSUM recycling at the end of a tile.
                fr = {0: mk_fraw(0)}
                fm = {0: mk_fmix(0)}
                for c in range(4):
                    fr[0](0, c)
                for c in range(4):
                    fr[0](1, c)
                fm[0](0, 0)
                fm[0](0, 1)
                for i in range(NT + 1):
                    tm = i if i < NT else None
                    tv = i - 1 if i >= 1 else None
                    av = mk_av(tv) if tv is not None else None
                    avq = [0, 1, 2, 3] if tv is not None else []
                    if tm is None:           # last iteration: drain
                        while avq:
                            av(avq.pop(0))
                        emit_outproj(tv)
                        continue
                    m2 = mk_m2(tm)
                    for k in range(8):
                        rt, rj = (tm, k + 2) if k < 6 else (tm + 1, k - 6)
                        if rt < NT and rj == 0:
                            fr[rt] = mk_fraw(rt)
                            fr.pop(rt - 1, None)
                        mt_, mj = (tm, k + 1) if k < 7 else (tm + 1, 0)
                        if mt_ < NT and mj == 0:
                            fm[mt_] = mk_fmix(mt_)
                            fm.pop(mt_ - 1, None)
                        raw = (lambda c: fr[rt](rj, c)) if rt < NT else (lambda c: None)
                        mix = (lambda h: fm[mt_](mj, h)) if mt_ < NT else (lambda h: None)
                        # interleave so no two psB raw allocs are adjacent in
                        # the PE queue: the ring recycles behind other work
                        raw(0)
                        raw(1)
                        mix(0)
                        raw(2)
                        m2h = m2(k)
                        m2h(0)
                        raw(3)
                        mix(1)
                        m2h(1)
                        if avq and k % 2 == 1:
                            av(avq.pop(0))
                        elif vfill and tv is None:
                            for _ in range(3 if k % 2 == 1 else 2):
                                if vfill:
                                    vfill.pop(0)()
                    while avq:
                        av(avq.pop(0))
                    if tv is not None:
                        emit_outproj(tv)

    nc.compile()
    return nc


_NC_CACHE = None


def _get_nc():
    global _NC_CACHE
    if _NC_CACHE is None:
        _NC_CACHE = build_nc()
    return _NC_CACHE


def _host_prep(w_qkv, b_qkv, th1, th2, w_out, b_out):
    bf = ml_dtypes.bfloat16
    scale = DH ** -0.5
    w_qkv = np.asarray(w_qkv, dtype=np.float32)
    wq = (w_qkv[:, 0:DIM] * scale).astype(bf)
    wk = w_qkv[:, DIM:2 * DIM].astype(bf)
    wv = w_qkv[:, 2 * DIM:3 * DIM].astype(bf)
    wo = np.asarray(w_out, dtype=np.float32).astype(bf)
    th1 = np.asarray(th1, dtype=np.float32)
    th2 = np.asarray(th2, dtype=np.float32)
    # mix1 template, rows (h,n16)=16h+n16, cols (g,n16):
    # t1t[h*16+n16, g*16+n16] = th1[g, h]
    t1t = np.zeros((128, 128), dtype=np.float32)
    for n16 in range(NG):
        t1t[n16::NG, n16::NG] = th1.T
    # mix2 template, (g, n16) partition order:
    # t2t[g*16+n16, g2*16+n16] = th2[g2, g]
    t2t = np.zeros((128, 128), dtype=np.float32)
    for n16 in range(NG):
        t2t[n16::NG, n16::NG] = th2.T
    bqkv = np.asarray(b_qkv, dtype=np.float32)
    bia = np.zeros((1, 4 * DIM), dtype=np.float32)
    bia[0, 0:DIM] = bqkv[0:DIM] * scale     # q bias scaled with w_q
    bia[0, DIM:3 * DIM] = bqkv[DIM:3 * DIM]
    bia[0, 3 * DIM:] = np.asarray(b_out, dtype=np.float32)
    return (wq, wk, wv, wo, t1t.astype(bf), t2t.astype(bf), bia.astype(bf))


def _core_inputs(x, prepped):
    """Per-core input maps.  x is the full [B, N, DIM] fp32 array."""
    bf = ml_dtypes.bfloat16
    wq, wk, wv, wo, t1t, t2t, bia = prepped
    xbf = np.asarray(x, dtype=np.float32).astype(bf)
    in_maps = []
    for c in range(NCORES):
        b, half = c // 2, c % 2
        # roll so this core's query rows come first; key order is a
        # permutation shared by K and V, which attention is invariant to.
        xc = np.ascontiguousarray(
            np.roll(xbf[b], -NLOC * half, axis=0))
        in_maps.append({
            "x": xc,
            "wq": wq, "wk": wk, "wv": wv, "wo": wo,
            "t1t": t1t, "t2t": t2t, "bia": bia,
            "eye": np.eye(128, dtype=np.float32).astype(bf),
        })
    return in_maps


def kernel(x, w_qkv, b_qkv, th1, th2, w_out, b_out):
    prepped = _host_prep(w_qkv, b_qkv, th1, th2, w_out, b_out)
    nc = _get_nc()
    in_maps = _core_inputs(x, prepped)
    res = run_bass_kernel_spmd(nc, in_maps, core_ids=list(range(NCORES)))
    out = np.empty((B, N, DIM), dtype=np.float32)
    for c in range(NCORES):
        b, half = c // 2, c % 2
        out[b, NLOC * half:NLOC * (half + 1), :] = res.results[c]["y"]
    return out
